# revision 20
# baseline (speedup 1.0000x reference)
"""Distributed TRN2 Bass kernel for OpenFold-style gated attention with pair bias.

Problem: B=4, Q=K=1024, H=8 heads, D=32, C=256 (all fp32):
    q = (q_x @ wq.T)/sqrt(D);  k = kv_x @ wk.T;  v = kv_x @ wv.T
    a = softmax(q k^T + mask_bias + pair_bias)   (softmax over K)
    o = (a v) * sigmoid(q_x @ wg.T + bg)
    out = o @ wo.T + bo

Sharding: 8 cores = (batch b, query-half qh).  Each core handles one batch's
full K and 512 queries across all 8 heads -> no collectives needed at all;
the host concatenates per-core outputs.

Device dataflow (all feature-on-partitions, no on-device transposes):
  - scores are computed directly transposed (s^T [k-part, q-free]); pair_bias
    is host-sharded to [h, k, q] and added on the DVE; softmax needs no
    max-subtraction (scores are O(6) here);
  - heads are processed in two groups of 4; the AV matmuls are column-packed
    (tile_position col groups) so one PSUM bank accumulates the stacked
    o^T for 4 heads [128=4x32d, 512q], and a u-weighted ones-vector matmul
    per head accumulates the softmax denominators into rows {0,32,64,96} of a
    second bank (u = exp(mask_bias) folded into v and the denominator makes
    mask_bias exact);
  - denominators are gathered to 4 partitions with one SBUF->SBUF DMA, one
    batched reciprocal, then broadcast back across partitions with a 0/1
    selector matmul; gating/normalization then run on stacked [128, 512]
    tiles and the output projection contracts the full 128-row halves.
"""

import numpy as np

H, D, C = 8, 32, 256
B, Q, K = 4, 1024, 1024
QL = 512  # queries per core
NCORES = 8
P = 128
NKC = K // P  # 8 k-chunks of 128

_CACHE = {}

# Stashed BassKernelResults from the most recent kernel() call (for profiling
# harnesses that want exec_time_ns / trace paths).
LAST_RESULTS = None


def _build_nc():
    from contextlib import ExitStack

    from concourse import bacc, mybir, tile

    f32 = mybir.dt.float32
    bf16 = mybir.dt.bfloat16
    EXP = mybir.ActivationFunctionType.Exp
    SIG = mybir.ActivationFunctionType.Sigmoid

    nc = bacc.Bacc("TRN2", target_bir_lowering=False, debug=False, num_devices=NCORES)

    CB = 5760  # bf16 constant-blob columns
    pbT_d = nc.dram_tensor("pbT", [H, K, QL], f32, kind="ExternalInput").ap()
    cb_d = nc.dram_tensor("cb", [P, CB], bf16, kind="ExternalInput").ap()
    cf_d = nc.dram_tensor("cf", [P, 12], f32, kind="ExternalInput").ap()
    out_d = nc.dram_tensor("out", [C, QL], f32, kind="ExternalOutput").ap()

    with tile.TileContext(nc) as tc, ExitStack() as ctx:
        # ---- persistent tiles -------------------------------------------
        cp = ctx.enter_context(tc.tile_pool(name="const", bufs=1))

        def ptile(shape, dtype, name):
            return cp.tile(shape, dtype, name=name, tag=name)

        cb_sb = ptile([P, CB], bf16, "cb_sb")
        cf_sb = ptile([P, 12], f32, "cf_sb")

        def cbv(lo, hi, a=None):
            v = cb_sb[:, lo:hi]
            return v.rearrange("p (a b) -> p a b", a=a) if a else v

        wq_bf = cbv(0, 512, 2)        # [128, 2, 256]
        wk_bf = cbv(512, 1024, 2)
        wv_bf = cbv(1024, 1536, 2)
        wg_bf = cbv(1536, 2048, 2)
        woB_bf = cbv(2048, 2560, 2)   # [hd-in-half, half t4, c]
        qx_bf = cbv(2560, 3584, 2)    # [128, 2, 512]
        kv_bf = cbv(3584, 5632, 2)    # [128, 2, 1024]
        e4_bf = cb_sb[0:4, 5632:5760]  # [4, 128]
        bgT_sb = cf_sb[:, 0:2]
        mbT_sb = cf_sb[:, 2:2 + NKC]
        boT_sb = cf_sb[:, 10:12]
        u_sb = ptile([P, NKC], f32, "u_sb")
        u_bf = ptile([P, NKC], bf16, "u_bf")
        ident_bf = ptile([P, P], bf16, "ident_bf")

        qT_bf = ptile([P, 2, QL], bf16, "qT_bf")  # [hd-part, t, q]
        kT_bf = ptile([P, 2, K], bf16, "kT_bf")  # [hd-part, t, k]
        v1_bf = ptile([P, NKC, C], bf16, "v1_bf")  # v * u, [k-part, chunk, hd]
        g_bf = ptile([P, 2, QL], bf16, "g_bf")  # sigmoid gate, stacked halves
        o4_sb = ptile([P, 2, QL], f32, "o4_sb")  # unnormalized o^T halves
        og_bf = ptile([P, 2, QL], bf16, "og_bf")  # gated+normalized o^T
        den_sb = ptile([P, 2, QL], f32, "den_sb")  # denom rows {0,32,64,96}

        nc.sync.dma_start(out=cb_sb[:, 0:2880], in_=cb_d[:, 0:2880])
        nc.scalar.dma_start(out=cb_sb[:, 2880:CB], in_=cb_d[:, 2880:CB])
        nc.scalar.dma_start(out=cf_sb[:], in_=cf_d[:])
        from concourse.masks import make_identity

        make_identity(nc, ident_bf[:])
        nc.scalar.activation(u_sb[:], mbT_sb[:], EXP)
        nc.vector.tensor_copy(u_bf[:], u_sb[:])

        # ---- stage 1: projections ---------------------------------------
        with tc.tile_pool(name="ps1", bufs=3, space="PSUM") as ps1:
            # qT[f, q] / kT[f, k] for hd-halves t
            for t in range(2):
                ps = ps1.tile([P, QL], f32, tag="ps1")
                for ci in range(2):
                    nc.tensor.matmul(
                        ps[:],
                        lhsT=wq_bf[:, ci, t * P:(t + 1) * P],
                        rhs=qx_bf[:, ci, :],
                        start=(ci == 0),
                        stop=(ci == 1),
                    )
                nc.vector.tensor_copy(qT_bf[:, t, :], ps[:])

            for t in range(2):
                for fc in range(2):
                    ps = ps1.tile([P, QL], f32, tag="ps1")
                    for ci in range(2):
                        nc.tensor.matmul(
                            ps[:],
                            lhsT=wk_bf[:, ci, t * P:(t + 1) * P],
                            rhs=kv_bf[:, ci, fc * QL:(fc + 1) * QL],
                            start=(ci == 0),
                            stop=(ci == 1),
                        )
                    nc.vector.tensor_copy(kT_bf[:, t, fc * QL:(fc + 1) * QL], ps[:])

            # v per k-chunk, scaled per-partition by u = exp(mask_bias)
            for j in range(NKC):
                ps = ps1.tile([P, C], f32, tag="ps1")
                for ci in range(2):
                    nc.tensor.matmul(
                        ps[:],
                        lhsT=kv_bf[:, ci, j * P:(j + 1) * P],
                        rhs=wv_bf[:, ci, :],
                        start=(ci == 0),
                        stop=(ci == 1),
                    )
                nc.scalar.activation(
                    v1_bf[:, j, :], ps[:], mybir.ActivationFunctionType.Copy,
                    bias=0.0, scale=u_sb[:, j:j + 1],
                )

            # gate halves: g = sigmoid(wg x + bg), stacked [128=4 heads x 32d]
            for t in range(2):
                ps = ps1.tile([P, QL], f32, tag="ps1")
                for ci in range(2):
                    nc.tensor.matmul(
                        ps[:],
                        lhsT=wg_bf[:, ci, t * P:(t + 1) * P],
                        rhs=qx_bf[:, ci, :],
                        start=(ci == 0),
                        stop=(ci == 1),
                    )
                nc.scalar.activation(
                    g_bf[:, t, :], ps[:], SIG, bias=bgT_sb[:, t:t + 1]
                )

        # ---- stage 2: attention, 2 groups of 4 column-packed heads ------
        with tc.tile_pool(name="pb", bufs=4) as pb_pool, tc.tile_pool(
            name="pp", bufs=4
        ) as p_pool, tc.tile_pool(name="nrm", bufs=2) as nrm, tc.tile_pool(
            name="ps_s", bufs=3, space="PSUM"
        ) as ps_s, tc.tile_pool(name="ps_o", bufs=1, space="PSUM") as ps_o, tc.tile_pool(
            name="ps_d", bufs=1, space="PSUM"
        ) as ps_d:
            ps_rb = ps_d
            for t4 in range(2):
                o_ps = ps_o.tile([P, QL], f32, tag="ps_o")
                d_ps = ps_d.tile([P, QL], f32, tag="ps_d")
                for j in range(NKC):
                    if j % 2 == 0:
                        jj = j // 2
                        pbt = pb_pool.tile([P, 2, 4, QL], bf16, tag="pb")
                        for h4 in range(4):
                            nc.gpsimd.dma_start(
                                out=pbt[:, :, h4, :],
                                in_=pbT_d[
                                    t4 * 4 + h4, 2 * jj * P:(2 * jj + 2) * P, :
                                ].rearrange("(j p) q -> p j q", p=P),
                            )
                    for pair in range(2):
                        h0 = 2 * pair  # heads (h0, h0+1) within the group
                        pr0, pr1 = h0 * D, (h0 + 1) * D
                        s2 = ps_s.tile([P, 2 * QL], f32, tag="ps_s")
                        on_pe = (j % 4 == 3)  # rebalance: 1/4 of adds via PE
                        for hh, pr in ((0, pr0), (1, pr1)):
                            sl = s2[:, hh * QL:(hh + 1) * QL]
                            if on_pe:
                                nc.tensor.matmul(
                                    sl,
                                    lhsT=ident_bf[:],
                                    rhs=pbt[:, j % 2, h0 + hh, :],
                                    start=True,
                                    stop=False,
                                    skip_group_check=True,
                                )
                            nc.tensor.matmul(
                                sl,
                                lhsT=kT_bf[pr:pr + D, t4, j * P:(j + 1) * P],
                                rhs=qT_bf[pr:pr + D, t4, :],
                                start=not on_pe,
                                stop=True,
                                tile_position=(pr, 0),
                                skip_group_check=True,
                            )
                        if not on_pe:
                            nc.vector.tensor_add(
                                s2[:],
                                s2[:],
                                pbt[:, j % 2, h0:h0 + 2, :].rearrange("p a b -> p (a b)"),
                            )
                        p2 = p_pool.tile([P, 2 * QL], bf16, tag="p2")
                        nc.scalar.activation(p2[:], s2[:], EXP)
                        for hh in range(2):
                            h4 = h0 + hh  # head index within group
                            co = h4 * D
                            nc.tensor.matmul(
                                o_ps[co:co + D, :],
                                lhsT=v1_bf[
                                    :, j, (t4 * 4 + h4) * D:(t4 * 4 + h4 + 1) * D
                                ],
                                rhs=p2[:, hh * QL:(hh + 1) * QL],
                                start=(j == 0),
                                stop=(j == NKC - 1),
                                tile_position=(0, co),
                                skip_group_check=True,
                            )
                            dco = ((h4 + 2) % 4) * D
                            nc.tensor.matmul(
                                d_ps[dco:dco + 1, :],
                                lhsT=u_bf[:, j:j + 1],
                                rhs=p2[:, hh * QL:(hh + 1) * QL],
                                start=(j == 0),
                                stop=(j == NKC - 1),
                                tile_position=(0, dco),
                                skip_group_check=True,
                            )
                # drain this group's AV/den PSUM then normalize inline so it
                # overlaps the next group's compute
                nc.vector.tensor_copy(o4_sb[:, t4, :], o_ps[:])
                nc.vector.tensor_copy(den_sb[:, t4, :], d_ps[:])
                recd_in = nrm.tile([4, QL], f32, tag="recd_in")
                nc.sync.dma_start(
                    out=recd_in[:],
                    in_=den_sb[:, t4, :].rearrange("(a b) q -> a b q", b=D)[:, 0, :],
                )
                recd = nrm.tile([4, QL], f32, tag="recd")
                nc.vector.reciprocal(recd[:], recd_in[:])
                recd_bf = nrm.tile([4, QL], bf16, tag="recd_bf")
                nc.vector.tensor_copy(recd_bf[:], recd[:])
                rb = ps_rb.tile([P, QL], f32, tag="ps_d", name="rb")
                nc.tensor.matmul(
                    rb[:], lhsT=e4_bf[:], rhs=recd_bf[:], start=True, stop=True
                )
                ge = nrm.tile([P, QL], bf16, tag="ge")
                nc.vector.tensor_mul(ge[:], g_bf[:, t4, :], rb[:])
                nc.vector.tensor_mul(og_bf[:, t4, :], o4_sb[:, t4, :], ge[:])

        # ---- stage 3: output projection ---------------------------------
        with tc.tile_pool(
            name="ps_out", bufs=2, space="PSUM"
        ) as ps_out_pool, tc.tile_pool(name="sb3", bufs=2) as sb3:
            pss = []
            for t in range(2):
                ps = ps_out_pool.tile([P, QL], f32, tag="ps_out")
                pss.append(ps)
                for t4 in range(2):
                    nc.tensor.matmul(
                        ps[:],
                        lhsT=woB_bf[:, t4, t * P:(t + 1) * P],
                        rhs=og_bf[:, t4, :],
                        start=(t4 == 0),
                        stop=(t4 == 1),
                    )
            for t in range(2):
                o_out = sb3.tile([P, QL], f32, tag="o_out")
                nc.vector.tensor_scalar_add(o_out[:], pss[t][:], boT_sb[:, t:t + 1])
                nc.sync.dma_start(out=out_d[t * P:(t + 1) * P, :], in_=o_out[:])

    nc.compile()
    return nc


def _get_nc():
    if "nc" not in _CACHE:
        _CACHE["nc"] = _build_nc()
    return _CACHE["nc"]


def _make_in_maps(q_x, kv_x, mask_bias, pair_bias, wq, wk, wv, wg, bg, wo, bo):
    f = np.float32
    q_x = np.asarray(q_x, f)
    kv_x = np.asarray(kv_x, f)
    mask_bias = np.asarray(mask_bias, f)
    pair_bias = np.asarray(pair_bias, f)
    wq = np.asarray(wq, f)
    wk = np.asarray(wk, f)
    wv = np.asarray(wv, f)
    wg = np.asarray(wg, f)
    bg = np.asarray(bg, f)
    wo = np.asarray(wo, f)
    bo = np.asarray(bo, f)

    import ml_dtypes
    bf = ml_dtypes.bfloat16

    def part_major(x, cols):  # [256, cols] -> [128, 2, cols] partition-major
        return x.reshape(2, P, cols).transpose(1, 0, 2)

    CB = 5760
    cb = np.zeros((P, CB), bf)
    cb[:, 0:512] = part_major((wq / np.sqrt(D)).T.astype(bf), C).reshape(P, 512)
    cb[:, 512:1024] = part_major(wk.T.astype(bf), C).reshape(P, 512)
    cb[:, 1024:1536] = part_major(wv.T.astype(bf), C).reshape(P, 512)
    cb[:, 1536:2048] = part_major(wg.T.astype(bf), C).reshape(P, 512)
    cb[:, 2048:2560] = (
        wo.T.reshape(2, P, C).transpose(1, 0, 2).astype(bf).reshape(P, 512)
    )
    # den rows land at partition ((i+2)%4)*32 -> selector maps row i to that head
    e4 = np.zeros((4, P), bf)
    for i in range(4):
        e4[i, ((i + 2) % 4) * D:(((i + 2) % 4) + 1) * D] = 1.0
    cb[0:4, 5632:5760] = e4
    cf = np.zeros((P, 12), np.float32)
    cf[:, 0:2] = bg.reshape(2, P).T
    cf[:, 10:12] = bo.reshape(2, P).T

    in_maps = []
    for c in range(NCORES):
        b, qh = c // 2, c % 2
        q0 = qh * QL
        cbc = cb.copy()
        cbc[:, 2560:3584] = part_major(
            q_x[b, q0:q0 + QL, :].T.astype(bf), QL
        ).reshape(P, 1024)
        cbc[:, 3584:5632] = part_major(kv_x[b].T.astype(bf), K).reshape(P, 2048)
        cfc = cf.copy()
        cfc[:, 2:2 + NKC] = mask_bias[b, 0, 0].reshape(NKC, P).T
        in_maps.append(
            {
                "pbT": np.ascontiguousarray(
                    pair_bias[b, :, q0:q0 + QL, :].transpose(0, 2, 1)
                ),
                "cb": cbc,
                "cf": cfc,
            }
        )
    return in_maps


def kernel(q_x, kv_x, mask_bias, pair_bias, wq, wk, wv, wg, bg, wo, bo):
    global LAST_RESULTS
    from concourse.bass_utils import run_bass_kernel_spmd

    nc = _get_nc()
    in_maps = _make_in_maps(
        q_x, kv_x, mask_bias, pair_bias, wq, wk, wv, wg, bg, wo, bo
    )
    res = run_bass_kernel_spmd(nc, in_maps, core_ids=list(range(NCORES)))
    LAST_RESULTS = res

    out = np.empty((B, Q, C), np.float32)
    for c in range(NCORES):
        b, qh = c // 2, c % 2
        out[b, qh * QL:(qh + 1) * QL, :] = res.results[c]["out"].T
    return out


# revision 21
# speedup vs baseline: 1.0946x; 1.0946x over previous
"""Distributed TRN2 Bass kernel for OpenFold-style gated attention with pair bias.

Problem: B=4, Q=K=1024, H=8 heads, D=32, C=256 (all fp32):
    q = (q_x @ wq.T)/sqrt(D);  k = kv_x @ wk.T;  v = kv_x @ wv.T
    a = softmax(q k^T + mask_bias + pair_bias)   (softmax over K)
    o = (a v) * sigmoid(q_x @ wg.T + bg)
    out = o @ wo.T + bo

Sharding: 8 cores = (batch b, query-half qh).  Each core handles one batch's
full K and 512 queries across all 8 heads -> no collectives needed at all;
the host concatenates per-core outputs.

Device dataflow (all feature-on-partitions, no on-device transposes):
  - scores are computed directly transposed (s^T [k-part, q-free]); pair_bias
    is host-sharded to [h, k, q] and added on the DVE; softmax needs no
    max-subtraction (scores are O(6) here);
  - heads are processed in two groups of 4; the AV matmuls are column-packed
    (tile_position col groups) so one PSUM bank accumulates the stacked
    o^T for 4 heads [128=4x32d, 512q], and a u-weighted ones-vector matmul
    per head accumulates the softmax denominators into rows {0,32,64,96} of a
    second bank (u = exp(mask_bias) folded into v and the denominator makes
    mask_bias exact);
  - denominators are gathered to 4 partitions with one SBUF->SBUF DMA, one
    batched reciprocal, then broadcast back across partitions with a 0/1
    selector matmul; gating/normalization then run on stacked [128, 512]
    tiles and the output projection contracts the full 128-row halves.
"""

import numpy as np

H, D, C = 8, 32, 256
B, Q, K = 4, 1024, 1024
QL = 512  # queries per core
NCORES = 8
P = 128
NKC = K // P  # 8 k-chunks of 128

_CACHE = {}

# Stashed BassKernelResults from the most recent kernel() call (for profiling
# harnesses that want exec_time_ns / trace paths).
LAST_RESULTS = None


def _build_nc():
    from contextlib import ExitStack

    from concourse import bacc, mybir, tile

    f32 = mybir.dt.float32
    bf16 = mybir.dt.bfloat16
    EXP = mybir.ActivationFunctionType.Exp
    SIG = mybir.ActivationFunctionType.Sigmoid

    nc = bacc.Bacc("TRN2", target_bir_lowering=False, debug=False, num_devices=NCORES)

    CB = 5760  # bf16 constant-blob columns
    pbT_d = nc.dram_tensor("pbT", [H, K, QL], f32, kind="ExternalInput").ap()
    cb_d = nc.dram_tensor("cb", [P, CB], bf16, kind="ExternalInput").ap()
    cf_d = nc.dram_tensor("cf", [P, 12], f32, kind="ExternalInput").ap()
    out_d = nc.dram_tensor("out", [C, QL], f32, kind="ExternalOutput").ap()

    with tile.TileContext(nc) as tc, ExitStack() as ctx:
        # ---- persistent tiles -------------------------------------------
        cp = ctx.enter_context(tc.tile_pool(name="const", bufs=1))

        def ptile(shape, dtype, name):
            return cp.tile(shape, dtype, name=name, tag=name)

        cb_sb = ptile([P, CB], bf16, "cb_sb")
        cf_sb = ptile([P, 12], f32, "cf_sb")

        def cbv(lo, hi, a=None):
            v = cb_sb[:, lo:hi]
            return v.rearrange("p (a b) -> p a b", a=a) if a else v

        wq_bf = cbv(0, 512, 2)        # [128, 2, 256]
        wk_bf = cbv(512, 1024, 2)
        wv_bf = cbv(1024, 1536, 2)
        wg_bf = cbv(1536, 2048, 2)
        woB_bf = cbv(2048, 2560, 2)   # [hd-in-half, half t4, c]
        qx_bf = cbv(2560, 3584, 2)    # [128, 2, 512]
        kv_bf = cbv(3584, 5632, 2)    # [128, 2, 1024]
        e4_bf = cb_sb[0:4, 5632:5760]  # [4, 128]
        bgT_sb = cf_sb[:, 0:2]
        mbT_sb = cf_sb[:, 2:2 + NKC]
        boT_sb = cf_sb[:, 10:12]
        u_sb = ptile([P, NKC], f32, "u_sb")
        u_bf = ptile([P, NKC], bf16, "u_bf")

        qT_bf = ptile([P, 2, QL], bf16, "qT_bf")  # [hd-part, t, q]
        kT_bf = ptile([P, 2, K], bf16, "kT_bf")  # [hd-part, t, k]
        v1_bf = ptile([P, NKC, C], bf16, "v1_bf")  # v * u, [k-part, chunk, hd]
        g_bf = ptile([P, 2, QL], bf16, "g_bf")  # sigmoid gate, stacked halves
        o4_sb = ptile([P, 2, QL], f32, "o4_sb")  # unnormalized o^T halves
        og_bf = ptile([P, 2, QL], bf16, "og_bf")  # gated+normalized o^T
        den_sb = ptile([P, 2, QL], f32, "den_sb")  # denom rows {0,32,64,96}

        nc.sync.dma_start(out=cb_sb[:, 0:2880], in_=cb_d[:, 0:2880])
        nc.scalar.dma_start(out=cb_sb[:, 2880:CB], in_=cb_d[:, 2880:CB])
        nc.scalar.dma_start(out=cf_sb[:], in_=cf_d[:])
        nc.scalar.activation(u_sb[:], mbT_sb[:], EXP)
        nc.vector.tensor_copy(u_bf[:], u_sb[:])

        # ---- stage 1: projections ---------------------------------------
        with tc.tile_pool(name="ps1", bufs=3, space="PSUM") as ps1:
            # qT[f, q] / kT[f, k] for hd-halves t
            for t in range(2):
                ps = ps1.tile([P, QL], f32, tag="ps1")
                for ci in range(2):
                    nc.tensor.matmul(
                        ps[:],
                        lhsT=wq_bf[:, ci, t * P:(t + 1) * P],
                        rhs=qx_bf[:, ci, :],
                        start=(ci == 0),
                        stop=(ci == 1),
                    )
                nc.vector.tensor_copy(qT_bf[:, t, :], ps[:])

            for t in range(2):
                for fc in range(2):
                    ps = ps1.tile([P, QL], f32, tag="ps1")
                    for ci in range(2):
                        nc.tensor.matmul(
                            ps[:],
                            lhsT=wk_bf[:, ci, t * P:(t + 1) * P],
                            rhs=kv_bf[:, ci, fc * QL:(fc + 1) * QL],
                            start=(ci == 0),
                            stop=(ci == 1),
                        )
                    nc.vector.tensor_copy(kT_bf[:, t, fc * QL:(fc + 1) * QL], ps[:])

            # v per k-chunk, scaled per-partition by u = exp(mask_bias)
            for j in range(NKC):
                ps = ps1.tile([P, C], f32, tag="ps1")
                for ci in range(2):
                    nc.tensor.matmul(
                        ps[:],
                        lhsT=kv_bf[:, ci, j * P:(j + 1) * P],
                        rhs=wv_bf[:, ci, :],
                        start=(ci == 0),
                        stop=(ci == 1),
                    )
                nc.scalar.activation(
                    v1_bf[:, j, :], ps[:], mybir.ActivationFunctionType.Copy,
                    bias=0.0, scale=u_sb[:, j:j + 1],
                )

            # gate halves: g = sigmoid(wg x + bg), stacked [128=4 heads x 32d]
            for t in range(2):
                ps = ps1.tile([P, QL], f32, tag="ps1")
                for ci in range(2):
                    nc.tensor.matmul(
                        ps[:],
                        lhsT=wg_bf[:, ci, t * P:(t + 1) * P],
                        rhs=qx_bf[:, ci, :],
                        start=(ci == 0),
                        stop=(ci == 1),
                    )
                nc.scalar.activation(
                    g_bf[:, t, :], ps[:], SIG, bias=bgT_sb[:, t:t + 1]
                )

        # ---- stage 2: attention, 2 groups of 4 column-packed heads ------
        with tc.tile_pool(name="pb", bufs=4) as pb_pool, tc.tile_pool(
            name="pp", bufs=4
        ) as p_pool, tc.tile_pool(name="nrm", bufs=2) as nrm, tc.tile_pool(
            name="ps_s", bufs=3, space="PSUM"
        ) as ps_s, tc.tile_pool(name="ps_o", bufs=1, space="PSUM") as ps_o, tc.tile_pool(
            name="ps_d", bufs=1, space="PSUM"
        ) as ps_d:
            ps_rb = ps_d
            for t4 in range(2):
                o_ps = ps_o.tile([P, QL], f32, tag="ps_o")
                d_ps = ps_d.tile([P, QL], f32, tag="ps_d")
                for j in range(NKC):
                    if j % 2 == 0:
                        jj = j // 2
                        pbt = pb_pool.tile([P, 2, 4, QL], bf16, tag="pb")
                        for h4 in range(4):
                            nc.gpsimd.dma_start(
                                out=pbt[:, :, h4, :],
                                in_=pbT_d[
                                    t4 * 4 + h4, 2 * jj * P:(2 * jj + 2) * P, :
                                ].rearrange("(j p) q -> p j q", p=P),
                            )
                    for pair in range(2):
                        h0 = 2 * pair  # heads (h0, h0+1) within the group
                        pr0, pr1 = h0 * D, (h0 + 1) * D
                        s2 = ps_s.tile([P, 2 * QL], f32, tag="ps_s")
                        for hh, pr in ((0, pr0), (1, pr1)):
                            nc.tensor.matmul(
                                s2[:, hh * QL:(hh + 1) * QL],
                                lhsT=kT_bf[pr:pr + D, t4, j * P:(j + 1) * P],
                                rhs=qT_bf[pr:pr + D, t4, :],
                                start=True,
                                stop=True,
                                tile_position=(pr, 0),
                            )
                        nc.vector.tensor_add(
                            s2[:],
                            s2[:],
                            pbt[:, j % 2, h0:h0 + 2, :].rearrange("p a b -> p (a b)"),
                        )
                        p2 = p_pool.tile([P, 2 * QL], bf16, tag="p2")
                        nc.scalar.activation(p2[:], s2[:], EXP)
                        for hh in range(2):
                            h4 = h0 + hh  # head index within group
                            co = h4 * D
                            nc.tensor.matmul(
                                o_ps[co:co + D, :],
                                lhsT=v1_bf[
                                    :, j, (t4 * 4 + h4) * D:(t4 * 4 + h4 + 1) * D
                                ],
                                rhs=p2[:, hh * QL:(hh + 1) * QL],
                                start=(j == 0),
                                stop=(j == NKC - 1),
                                tile_position=(0, co),
                                skip_group_check=True,
                            )
                            dco = ((h4 + 2) % 4) * D
                            nc.tensor.matmul(
                                d_ps[dco:dco + 1, :],
                                lhsT=u_bf[:, j:j + 1],
                                rhs=p2[:, hh * QL:(hh + 1) * QL],
                                start=(j == 0),
                                stop=(j == NKC - 1),
                                tile_position=(0, dco),
                                skip_group_check=True,
                            )
                # drain this group's AV/den PSUM then normalize inline so it
                # overlaps the next group's compute
                nc.vector.tensor_copy(o4_sb[:, t4, :], o_ps[:])
                nc.vector.tensor_copy(den_sb[:, t4, :], d_ps[:])
                recd_in = nrm.tile([4, QL], f32, tag="recd_in")
                nc.sync.dma_start(
                    out=recd_in[:],
                    in_=den_sb[:, t4, :].rearrange("(a b) q -> a b q", b=D)[:, 0, :],
                )
                recd = nrm.tile([4, QL], f32, tag="recd")
                nc.vector.reciprocal(recd[:], recd_in[:])
                recd_bf = nrm.tile([4, QL], bf16, tag="recd_bf")
                nc.vector.tensor_copy(recd_bf[:], recd[:])
                rb = ps_rb.tile([P, QL], f32, tag="ps_d", name="rb")
                nc.tensor.matmul(
                    rb[:], lhsT=e4_bf[:], rhs=recd_bf[:], start=True, stop=True
                )
                ge = nrm.tile([P, QL], bf16, tag="ge")
                nc.vector.tensor_mul(ge[:], g_bf[:, t4, :], rb[:])
                nc.vector.tensor_mul(og_bf[:, t4, :], o4_sb[:, t4, :], ge[:])

        # ---- stage 3: output projection ---------------------------------
        with tc.tile_pool(
            name="ps_out", bufs=2, space="PSUM"
        ) as ps_out_pool, tc.tile_pool(name="sb3", bufs=2) as sb3:
            pss = []
            for t in range(2):
                ps = ps_out_pool.tile([P, QL], f32, tag="ps_out")
                pss.append(ps)
                for t4 in range(2):
                    nc.tensor.matmul(
                        ps[:],
                        lhsT=woB_bf[:, t4, t * P:(t + 1) * P],
                        rhs=og_bf[:, t4, :],
                        start=(t4 == 0),
                        stop=(t4 == 1),
                    )
            for t in range(2):
                o_out = sb3.tile([P, QL], f32, tag="o_out")
                nc.vector.tensor_scalar_add(o_out[:], pss[t][:], boT_sb[:, t:t + 1])
                nc.sync.dma_start(out=out_d[t * P:(t + 1) * P, :], in_=o_out[:])

    nc.compile()
    return nc


def _get_nc():
    if "nc" not in _CACHE:
        _CACHE["nc"] = _build_nc()
    return _CACHE["nc"]


def _make_in_maps(q_x, kv_x, mask_bias, pair_bias, wq, wk, wv, wg, bg, wo, bo):
    f = np.float32
    q_x = np.asarray(q_x, f)
    kv_x = np.asarray(kv_x, f)
    mask_bias = np.asarray(mask_bias, f)
    pair_bias = np.asarray(pair_bias, f)
    wq = np.asarray(wq, f)
    wk = np.asarray(wk, f)
    wv = np.asarray(wv, f)
    wg = np.asarray(wg, f)
    bg = np.asarray(bg, f)
    wo = np.asarray(wo, f)
    bo = np.asarray(bo, f)

    import ml_dtypes
    bf = ml_dtypes.bfloat16

    def part_major(x, cols):  # [256, cols] -> [128, 2, cols] partition-major
        return x.reshape(2, P, cols).transpose(1, 0, 2)

    CB = 5760
    cb = np.zeros((P, CB), bf)
    cb[:, 0:512] = part_major((wq / np.sqrt(D)).T.astype(bf), C).reshape(P, 512)
    cb[:, 512:1024] = part_major(wk.T.astype(bf), C).reshape(P, 512)
    cb[:, 1024:1536] = part_major(wv.T.astype(bf), C).reshape(P, 512)
    cb[:, 1536:2048] = part_major(wg.T.astype(bf), C).reshape(P, 512)
    cb[:, 2048:2560] = (
        wo.T.reshape(2, P, C).transpose(1, 0, 2).astype(bf).reshape(P, 512)
    )
    # den rows land at partition ((i+2)%4)*32 -> selector maps row i to that head
    e4 = np.zeros((4, P), bf)
    for i in range(4):
        e4[i, ((i + 2) % 4) * D:(((i + 2) % 4) + 1) * D] = 1.0
    cb[0:4, 5632:5760] = e4
    cf = np.zeros((P, 12), np.float32)
    cf[:, 0:2] = bg.reshape(2, P).T
    cf[:, 10:12] = bo.reshape(2, P).T

    in_maps = []
    for c in range(NCORES):
        b, qh = c // 2, c % 2
        q0 = qh * QL
        cbc = cb.copy()
        cbc[:, 2560:3584] = part_major(
            q_x[b, q0:q0 + QL, :].T.astype(bf), QL
        ).reshape(P, 1024)
        cbc[:, 3584:5632] = part_major(kv_x[b].T.astype(bf), K).reshape(P, 2048)
        cfc = cf.copy()
        cfc[:, 2:2 + NKC] = mask_bias[b, 0, 0].reshape(NKC, P).T
        in_maps.append(
            {
                "pbT": np.ascontiguousarray(
                    pair_bias[b, :, q0:q0 + QL, :].transpose(0, 2, 1)
                ),
                "cb": cbc,
                "cf": cfc,
            }
        )
    return in_maps


def kernel(q_x, kv_x, mask_bias, pair_bias, wq, wk, wv, wg, bg, wo, bo):
    global LAST_RESULTS
    from concourse.bass_utils import run_bass_kernel_spmd

    nc = _get_nc()
    in_maps = _make_in_maps(
        q_x, kv_x, mask_bias, pair_bias, wq, wk, wv, wg, bg, wo, bo
    )
    res = run_bass_kernel_spmd(nc, in_maps, core_ids=list(range(NCORES)))
    LAST_RESULTS = res

    out = np.empty((B, Q, C), np.float32)
    for c in range(NCORES):
        b, qh = c // 2, c % 2
        out[b, qh * QL:(qh + 1) * QL, :] = res.results[c]["out"].T
    return out


# revision 22
# speedup vs baseline: 1.1328x; 1.0349x over previous
"""Distributed TRN2 Bass kernel for OpenFold-style gated attention with pair bias.

Problem: B=4, Q=K=1024, H=8 heads, D=32, C=256 (all fp32):
    q = (q_x @ wq.T)/sqrt(D);  k = kv_x @ wk.T;  v = kv_x @ wv.T
    a = softmax(q k^T + mask_bias + pair_bias)   (softmax over K)
    o = (a v) * sigmoid(q_x @ wg.T + bg)
    out = o @ wo.T + bo

Sharding: 8 cores = (batch b, query-half qh).  Each core handles one batch's
full K and 512 queries across all 8 heads -> no collectives needed at all;
the host concatenates per-core outputs.

Device dataflow (all feature-on-partitions, no on-device transposes):
  - scores are computed directly transposed (s^T [k-part, q-free]); pair_bias
    is host-sharded to [h, k, q] and added on the DVE; softmax needs no
    max-subtraction (scores are O(6) here);
  - heads are processed in two groups of 4; the AV matmuls are column-packed
    (tile_position col groups) so one PSUM bank accumulates the stacked
    o^T for 4 heads [128=4x32d, 512q], and a u-weighted ones-vector matmul
    per head accumulates the softmax denominators into rows {0,32,64,96} of a
    second bank (u = exp(mask_bias) folded into v and the denominator makes
    mask_bias exact);
  - denominators are gathered to 4 partitions with one SBUF->SBUF DMA, one
    batched reciprocal, then broadcast back across partitions with a 0/1
    selector matmul; gating/normalization then run on stacked [128, 512]
    tiles and the output projection contracts the full 128-row halves.
"""

import numpy as np

H, D, C = 8, 32, 256
B, Q, K = 4, 1024, 1024
QL = 512  # queries per core
NCORES = 8
P = 128
NKC = K // P  # 8 k-chunks of 128

_CACHE = {}

# Stashed BassKernelResults from the most recent kernel() call (for profiling
# harnesses that want exec_time_ns / trace paths).
LAST_RESULTS = None


def _build_nc():
    from contextlib import ExitStack

    from concourse import bacc, mybir, tile

    f32 = mybir.dt.float32
    bf16 = mybir.dt.bfloat16
    EXP = mybir.ActivationFunctionType.Exp
    SIG = mybir.ActivationFunctionType.Sigmoid

    nc = bacc.Bacc("TRN2", target_bir_lowering=False, debug=False, num_devices=NCORES)

    CB = 5760  # bf16 constant-blob columns
    pbT_d = nc.dram_tensor("pbT", [H, K, QL], f32, kind="ExternalInput").ap()
    cb_d = nc.dram_tensor("cb", [P, CB], bf16, kind="ExternalInput").ap()
    cf_d = nc.dram_tensor("cf", [P, 12], f32, kind="ExternalInput").ap()
    out_d = nc.dram_tensor("out", [C, QL], f32, kind="ExternalOutput").ap()

    with tile.TileContext(nc) as tc, ExitStack() as ctx:
        # ---- persistent tiles -------------------------------------------
        cp = ctx.enter_context(tc.tile_pool(name="const", bufs=1))

        def ptile(shape, dtype, name):
            return cp.tile(shape, dtype, name=name, tag=name)

        cb_sb = ptile([P, CB], bf16, "cb_sb")
        cf_sb = ptile([P, 12], f32, "cf_sb")

        def cbv(lo, hi, a=None):
            v = cb_sb[:, lo:hi]
            return v.rearrange("p (a b) -> p a b", a=a) if a else v

        wq_bf = cbv(0, 512, 2)        # [128, 2, 256]
        wk_bf = cbv(512, 1024, 2)
        wv_bf = cbv(1024, 1536, 2)
        wg_bf = cbv(1536, 2048, 2)
        woB_bf = cbv(2048, 2560, 2)   # [hd-in-half, half t4, c]
        qx_bf = cbv(2560, 3584, 2)    # [128, 2, 512]
        kv_bf = cbv(3584, 5632, 2)    # [128, 2, 1024]
        e4_bf = cb_sb[0:4, 5632:5760]  # [4, 128]
        bgT_sb = cf_sb[:, 0:2]
        mbT_sb = cf_sb[:, 2:2 + NKC]
        boT_sb = cf_sb[:, 10:12]
        u_sb = ptile([P, NKC], f32, "u_sb")
        u_bf = ptile([P, NKC], bf16, "u_bf")

        qT_bf = ptile([P, 2, QL], bf16, "qT_bf")  # [hd-part, t, q]
        kT_bf = ptile([P, 2, K], bf16, "kT_bf")  # [hd-part, t, k]
        v1_bf = ptile([P, NKC, C], bf16, "v1_bf")  # v * u, [k-part, chunk, hd]
        g_bf = ptile([P, 2, QL], bf16, "g_bf")  # sigmoid gate, stacked halves
        o4_sb = ptile([P, 2, QL], f32, "o4_sb")  # unnormalized o^T halves
        og_bf = ptile([P, 2, QL], bf16, "og_bf")  # gated+normalized o^T
        den_sb = ptile([P, 2, QL], f32, "den_sb")  # denom rows {0,32,64,96}

        nc.sync.dma_start(out=cb_sb[:, 0:2880], in_=cb_d[:, 0:2880])
        nc.scalar.dma_start(out=cb_sb[:, 2880:CB], in_=cb_d[:, 2880:CB])
        nc.scalar.dma_start(out=cf_sb[:], in_=cf_d[:])
        nc.scalar.activation(u_sb[:], mbT_sb[:], EXP)
        nc.vector.tensor_copy(u_bf[:], u_sb[:])

        # ---- stage 1: projections ---------------------------------------
        with tc.tile_pool(name="ps1", bufs=3, space="PSUM") as ps1:
            # qT[f, q] / kT[f, k] for hd-halves t
            for t in range(2):
                ps = ps1.tile([P, QL], f32, tag="ps1")
                for ci in range(2):
                    nc.tensor.matmul(
                        ps[:],
                        lhsT=wq_bf[:, ci, t * P:(t + 1) * P],
                        rhs=qx_bf[:, ci, :],
                        start=(ci == 0),
                        stop=(ci == 1),
                    )
                nc.vector.tensor_copy(qT_bf[:, t, :], ps[:])

            for t in range(2):
                for fc in range(2):
                    ps = ps1.tile([P, QL], f32, tag="ps1")
                    for ci in range(2):
                        nc.tensor.matmul(
                            ps[:],
                            lhsT=wk_bf[:, ci, t * P:(t + 1) * P],
                            rhs=kv_bf[:, ci, fc * QL:(fc + 1) * QL],
                            start=(ci == 0),
                            stop=(ci == 1),
                        )
                    nc.vector.tensor_copy(kT_bf[:, t, fc * QL:(fc + 1) * QL], ps[:])

            # v per k-chunk, scaled per-partition by u = exp(mask_bias)
            for j in range(NKC):
                ps = ps1.tile([P, C], f32, tag="ps1")
                for ci in range(2):
                    nc.tensor.matmul(
                        ps[:],
                        lhsT=kv_bf[:, ci, j * P:(j + 1) * P],
                        rhs=wv_bf[:, ci, :],
                        start=(ci == 0),
                        stop=(ci == 1),
                    )
                nc.scalar.activation(
                    v1_bf[:, j, :], ps[:], mybir.ActivationFunctionType.Copy,
                    bias=0.0, scale=u_sb[:, j:j + 1],
                )

            # gate halves: g = sigmoid(wg x + bg), stacked [128=4 heads x 32d]
            for t in range(2):
                ps = ps1.tile([P, QL], f32, tag="ps1")
                for ci in range(2):
                    nc.tensor.matmul(
                        ps[:],
                        lhsT=wg_bf[:, ci, t * P:(t + 1) * P],
                        rhs=qx_bf[:, ci, :],
                        start=(ci == 0),
                        stop=(ci == 1),
                    )
                nc.scalar.activation(
                    g_bf[:, t, :], ps[:], SIG, bias=bgT_sb[:, t:t + 1]
                )

        # ---- stage 2: attention, 2 groups of 4 column-packed heads ------
        with tc.tile_pool(name="pb", bufs=6) as pb_pool, tc.tile_pool(
            name="pp", bufs=6
        ) as p_pool, tc.tile_pool(name="ss", bufs=3) as s_pool, tc.tile_pool(name="nrm", bufs=2) as nrm, tc.tile_pool(
            name="ps_s", bufs=3, space="PSUM"
        ) as ps_s, tc.tile_pool(name="ps_o", bufs=1, space="PSUM") as ps_o, tc.tile_pool(
            name="ps_d", bufs=1, space="PSUM"
        ) as ps_d:
            ps_rb = ps_d
            for t4 in range(2):
                o_ps = ps_o.tile([P, QL], f32, tag="ps_o")
                d_ps = ps_d.tile([P, QL], f32, tag="ps_d")
                for j in range(NKC):
                    if j % 2 == 0:
                        jj = j // 2
                        pbt = pb_pool.tile([P, 2, 4, QL], bf16, tag="pb")
                        for h4 in range(4):
                            nc.gpsimd.dma_start(
                                out=pbt[:, :, h4, :],
                                in_=pbT_d[
                                    t4 * 4 + h4, 2 * jj * P:(2 * jj + 2) * P, :
                                ].rearrange("(j p) q -> p j q", p=P),
                            )
                    for pair in range(2):
                        h0 = 2 * pair  # heads (h0, h0+1) within the group
                        pr0, pr1 = h0 * D, (h0 + 1) * D
                        s2 = ps_s.tile([P, 2 * QL], f32, tag="ps_s")
                        for hh, pr in ((0, pr0), (1, pr1)):
                            nc.tensor.matmul(
                                s2[:, hh * QL:(hh + 1) * QL],
                                lhsT=kT_bf[pr:pr + D, t4, j * P:(j + 1) * P],
                                rhs=qT_bf[pr:pr + D, t4, :],
                                start=True,
                                stop=True,
                                tile_position=(pr, 0),
                            )
                        s_sb = s_pool.tile([P, 2 * QL], f32, tag="s_sb")
                        nc.vector.tensor_add(
                            s_sb[:],
                            s2[:],
                            pbt[:, j % 2, h0:h0 + 2, :].rearrange("p a b -> p (a b)"),
                        )
                        p2 = p_pool.tile([P, 2 * QL], bf16, tag="p2")
                        nc.scalar.activation(p2[:], s_sb[:], EXP)
                        for hh in range(2):
                            h4 = h0 + hh  # head index within group
                            co = h4 * D
                            nc.tensor.matmul(
                                o_ps[co:co + D, :],
                                lhsT=v1_bf[
                                    :, j, (t4 * 4 + h4) * D:(t4 * 4 + h4 + 1) * D
                                ],
                                rhs=p2[:, hh * QL:(hh + 1) * QL],
                                start=(j == 0),
                                stop=(j == NKC - 1),
                                tile_position=(0, co),
                                skip_group_check=True,
                            )
                            dco = ((h4 + 2) % 4) * D
                            nc.tensor.matmul(
                                d_ps[dco:dco + 1, :],
                                lhsT=u_bf[:, j:j + 1],
                                rhs=p2[:, hh * QL:(hh + 1) * QL],
                                start=(j == 0),
                                stop=(j == NKC - 1),
                                tile_position=(0, dco),
                                skip_group_check=True,
                            )
                # drain this group's AV/den PSUM then normalize inline so it
                # overlaps the next group's compute
                nc.vector.tensor_copy(o4_sb[:, t4, :], o_ps[:])
                nc.vector.tensor_copy(den_sb[:, t4, :], d_ps[:])
                recd_in = nrm.tile([4, QL], f32, tag="recd_in")
                nc.sync.dma_start(
                    out=recd_in[:],
                    in_=den_sb[:, t4, :].rearrange("(a b) q -> a b q", b=D)[:, 0, :],
                )
                recd = nrm.tile([4, QL], f32, tag="recd")
                nc.vector.reciprocal(recd[:], recd_in[:])
                recd_bf = nrm.tile([4, QL], bf16, tag="recd_bf")
                nc.vector.tensor_copy(recd_bf[:], recd[:])
                rb = ps_rb.tile([P, QL], f32, tag="ps_d", name="rb")
                nc.tensor.matmul(
                    rb[:], lhsT=e4_bf[:], rhs=recd_bf[:], start=True, stop=True
                )
                ge = nrm.tile([P, QL], bf16, tag="ge")
                nc.vector.tensor_mul(ge[:], g_bf[:, t4, :], rb[:])
                nc.vector.tensor_mul(og_bf[:, t4, :], o4_sb[:, t4, :], ge[:])

        # ---- stage 3: output projection ---------------------------------
        with tc.tile_pool(
            name="ps_out", bufs=2, space="PSUM"
        ) as ps_out_pool, tc.tile_pool(name="sb3", bufs=2) as sb3:
            pss = []
            for t in range(2):
                ps = ps_out_pool.tile([P, QL], f32, tag="ps_out")
                pss.append(ps)
                for t4 in range(2):
                    nc.tensor.matmul(
                        ps[:],
                        lhsT=woB_bf[:, t4, t * P:(t + 1) * P],
                        rhs=og_bf[:, t4, :],
                        start=(t4 == 0),
                        stop=(t4 == 1),
                    )
            for t in range(2):
                o_out = sb3.tile([P, QL], f32, tag="o_out")
                nc.vector.tensor_scalar_add(o_out[:], pss[t][:], boT_sb[:, t:t + 1])
                nc.sync.dma_start(out=out_d[t * P:(t + 1) * P, :], in_=o_out[:])

    nc.compile()
    return nc


def _get_nc():
    if "nc" not in _CACHE:
        _CACHE["nc"] = _build_nc()
    return _CACHE["nc"]


def _make_in_maps(q_x, kv_x, mask_bias, pair_bias, wq, wk, wv, wg, bg, wo, bo):
    f = np.float32
    q_x = np.asarray(q_x, f)
    kv_x = np.asarray(kv_x, f)
    mask_bias = np.asarray(mask_bias, f)
    pair_bias = np.asarray(pair_bias, f)
    wq = np.asarray(wq, f)
    wk = np.asarray(wk, f)
    wv = np.asarray(wv, f)
    wg = np.asarray(wg, f)
    bg = np.asarray(bg, f)
    wo = np.asarray(wo, f)
    bo = np.asarray(bo, f)

    import ml_dtypes
    bf = ml_dtypes.bfloat16

    def part_major(x, cols):  # [256, cols] -> [128, 2, cols] partition-major
        return x.reshape(2, P, cols).transpose(1, 0, 2)

    CB = 5760
    cb = np.zeros((P, CB), bf)
    cb[:, 0:512] = part_major((wq / np.sqrt(D)).T.astype(bf), C).reshape(P, 512)
    cb[:, 512:1024] = part_major(wk.T.astype(bf), C).reshape(P, 512)
    cb[:, 1024:1536] = part_major(wv.T.astype(bf), C).reshape(P, 512)
    cb[:, 1536:2048] = part_major(wg.T.astype(bf), C).reshape(P, 512)
    cb[:, 2048:2560] = (
        wo.T.reshape(2, P, C).transpose(1, 0, 2).astype(bf).reshape(P, 512)
    )
    # den rows land at partition ((i+2)%4)*32 -> selector maps row i to that head
    e4 = np.zeros((4, P), bf)
    for i in range(4):
        e4[i, ((i + 2) % 4) * D:(((i + 2) % 4) + 1) * D] = 1.0
    cb[0:4, 5632:5760] = e4
    cf = np.zeros((P, 12), np.float32)
    cf[:, 0:2] = bg.reshape(2, P).T
    cf[:, 10:12] = bo.reshape(2, P).T

    in_maps = []
    for c in range(NCORES):
        b, qh = c // 2, c % 2
        q0 = qh * QL
        cbc = cb.copy()
        cbc[:, 2560:3584] = part_major(
            q_x[b, q0:q0 + QL, :].T.astype(bf), QL
        ).reshape(P, 1024)
        cbc[:, 3584:5632] = part_major(kv_x[b].T.astype(bf), K).reshape(P, 2048)
        cfc = cf.copy()
        cfc[:, 2:2 + NKC] = mask_bias[b, 0, 0].reshape(NKC, P).T
        in_maps.append(
            {
                "pbT": np.ascontiguousarray(
                    pair_bias[b, :, q0:q0 + QL, :].transpose(0, 2, 1)
                ),
                "cb": cbc,
                "cf": cfc,
            }
        )
    return in_maps


def kernel(q_x, kv_x, mask_bias, pair_bias, wq, wk, wv, wg, bg, wo, bo):
    global LAST_RESULTS
    from concourse.bass_utils import run_bass_kernel_spmd

    nc = _get_nc()
    in_maps = _make_in_maps(
        q_x, kv_x, mask_bias, pair_bias, wq, wk, wv, wg, bg, wo, bo
    )
    res = run_bass_kernel_spmd(nc, in_maps, core_ids=list(range(NCORES)))
    LAST_RESULTS = res

    out = np.empty((B, Q, C), np.float32)
    for c in range(NCORES):
        b, qh = c // 2, c % 2
        out[b, qh * QL:(qh + 1) * QL, :] = res.results[c]["out"].T
    return out
